# revision 28
# baseline (speedup 1.0000x reference)
"""Trainium2 Bass kernel for nn_BoxesFromMasks (per-frame segment bounding boxes).

Algorithm (per core, data-parallel over frames, TL=2 frames/core):
  Build per-pixel 64-bit one-hot bitmasks (2 u32 planes) of the instance id via
  the exponent-bit trick (ACT builds the f32 bit pattern of 2^k as an int, a
  second ACT converts value->u32, truncating out-of-range ids to 0):
    lo plane: id s in [0,32)  -> bit (31-s)
    hi plane: id s in [32,64) -> bit (s-32)
  Row masks:  OR-tree each 128-row chunk along columns (DVE), 16-wide leftovers
              folded once at extraction time.
  Col masks:  OR-accumulate chunks into acc[128,2,W]; pre-fold partitions
              128->64; DMA-transpose (u16); OR-tree the 64 contributors.
  Extraction (batched, no DRAM bounce): 16 u16 shift ops expand bits to
  E-tables, constant value-tables select coordinates via i16 mult/add, strided
  tensor_reduce min/max, one 3x128 transpose fold, and negative-stride output
  DMAs undo the bit-order permutation.
"""

import numpy as np

_T, _H, _W, _N = 16, 1024, 2048, 64
_NCORES = 8

_BUILD_CACHE = {}


def _build_program(TL, H, W, split_waits=True, reps=1):
    from contextlib import ExitStack

    import bass_rust
    import concourse.bass as bass
    import concourse.tile as tile
    import concourse.mybir as mybir
    from concourse.alu_op_type import AluOpType as Op

    f32 = mybir.dt.float32
    i32 = mybir.dt.int32
    u32 = mybir.dt.uint32
    u16 = mybir.dt.uint16
    i16 = mybir.dt.int16
    Copy = mybir.ActivationFunctionType.Copy
    X = mybir.AxisListType.X

    P = 128
    CH = H // P                   # row chunks per frame (8)
    KT = 4                        # transpose calls per frame (each 2048 u16 cols)
    MPER = 16                     # 128-col blocks per transpose call
    NSEG = 2                      # seg DMA splits per chunk
    BIG = 32767
    assert TL == 2 and CH == 8 and W == 2048

    # ---- constant value tables (i16) ----
    pp = np.arange(P)
    # Y: value v(p, c) = 128c + p ; table shape [P, 64(pl f j), CH, 2h]
    yv = (128 * np.arange(CH)[None, :] + pp[:, None]).astype(np.int64)   # [P, CH]
    ty_mb = np.broadcast_to((yv - BIG)[:, None, :, None],
                            (P, 64, CH, 2)).astype(np.int16)
    ty_p1 = np.broadcast_to((yv + 1)[:, None, :, None],
                            (P, 64, CH, 2)).astype(np.int16)
    # X: value v(q, klo, m) = klo*1024 + 64m + (q>>1) ; table [P, 64(pl f j), 32(klo m)]
    klo = np.arange(2)
    mm = np.arange(MPER)
    xv = ((klo[:, None] * 1024 + 64 * mm[None, :]).reshape(-1)[None, :]
          + (pp[:, None] // 2)).astype(np.int64)                         # [P, 32]
    tx_mb = np.broadcast_to((xv - BIG)[:, None, :], (P, 64, 32)).astype(np.int16)
    tx_p1 = np.broadcast_to((xv + 1)[:, None, :], (P, 64, 32)).astype(np.int16)

    tables = {"ty_mb": ty_mb, "ty_p1": ty_p1, "tx_mb": tx_mb, "tx_p1": tx_p1}

    nc = bass.Bass()
    seg_in = nc.dram_tensor("seg", [TL, H, W], i32, kind="ExternalInput")
    boxes_out = nc.dram_tensor("boxes", [TL, 64, 4], f32, kind="ExternalOutput")
    d_tabs = {n: nc.dram_tensor(n, list(t.shape), i16, kind="ExternalInput")
              for n, t in tables.items()}

    def dram_ap(t, offset_elems, dims):
        """Manual DRAM AP: dims = [(stride_elems, count), ...]."""
        a2 = t[:].copy()
        a2.offset = offset_elems
        a2.ap = bass_rust.VecI64Pair([[s, n] for s, n in dims])
        return a2

    with tile.TileContext(nc) as tc, ExitStack() as ctx:
        constp = ctx.enter_context(tc.tile_pool(name="consts", bufs=1))
        segp = ctx.enter_context(tc.tile_pool(name="segp", bufs=3))
        ep = ctx.enter_context(tc.tile_pool(name="ep", bufs=3))
        accp = ctx.enter_context(tc.tile_pool(name="accp", bufs=2))
        accTp = ctx.enter_context(tc.tile_pool(name="accTp", bufs=2))
        rmp = ctx.enter_context(tc.tile_pool(name="rmp", bufs=2))
        xp = ctx.enter_context(tc.tile_pool(name="xp", bufs=2))
        smallp = ctx.enter_context(tc.tile_pool(name="smallp", bufs=2))

        c_ty_mb = constp.tile([P, 64, CH, 2], i16)
        c_ty_p1 = constp.tile([P, 64, CH, 2], i16)
        c_tx_mb = constp.tile([P, 64, 32], i16)
        c_tx_p1 = constp.tile([P, 64, 32], i16)
        const_loaded = [False]

        def load_consts():
            if const_loaded[0]:
                return
            const_loaded[0] = True
            for t, n in [(c_ty_mb, "ty_mb"), (c_ty_p1, "ty_p1"),
                         (c_tx_mb, "tx_mb"), (c_tx_p1, "tx_p1")]:
                nc.scalar.dma_start(t[:], d_tabs[n][:])

        for _rep in range(reps):
            # rmask: [p, pl, f, c] u32 (pl-major for contiguous planes)
            rmask = rmp.tile([P, 2, TL, CH], u32, tag="rmask")
            # CMX: [q, pl, f, klo, m] u16 (compacted column masks)
            CMX = xp.tile([P, 2, TL, 2, MPER], u16, tag="cmx")

            # ================= main loop =================
            # seg tiles are created and their loads issued ahead of use so
            # next-frame loads precede this frame's transposes on the queue
            seg_tiles = {}

            def issue_seg(f, c):
                if f >= TL or (f, c) in seg_tiles:
                    return
                s = segp.tile([P, W], i32, tag="seg")
                nseg = 4 if (f == 0 and c == 0) else NSEG
                rows = P // nseg
                for k in range(nseg):
                    nc.sync.dma_start(
                        s[rows * k:rows * (k + 1), :],
                        seg_in[f, c * P + rows * k:c * P + rows * (k + 1), :])
                seg_tiles[(f, c)] = s

            for c2 in range(3):
                issue_seg(0, c2)
            load_consts()

            for f in range(TL):
                acc = accp.tile([P, 2, W], u32)
                prev_u = None
                for c in range(CH):
                    issue_seg(f, c)
                    s = seg_tiles.pop((f, c))

                    e = ep.tile([P, 2, W], i32)
                    # lo: bitpattern of 2^(31-s) = (158-s)<<23 ; hi: 2^(s-32) = (s+95)<<23
                    # first chunk of the kernel: split build/cast into partition
                    # halves so the pipeline fills ~4us sooner
                    halves = ([(0, 64), (64, 128)] if (f == 0 and c == 0)
                              else [(0, 128)])
                    for p0, p1 in halves:
                        nc.scalar.activation(e[p0:p1, 0, :], s[p0:p1], Copy,
                                             bias=1325400064.0, scale=-8388608.0)
                        nc.gpsimd.tensor_scalar(e[p0:p1, 1, :], s[p0:p1],
                                                8388608, 796917760,
                                                Op.mult, Op.add)
                        nc.scalar.activation(e[p0:p1].bitcast(u32),
                                             e[p0:p1].bitcast(f32), Copy)
                    u = e[:].bitcast(u32)  # cast in place

                    # column accumulate (DVE; only DVE has integer bitwise ops)
                    if c == 0:
                        prev_u = u
                    elif c == 1:
                        nc.vector.tensor_tensor(acc[:], u, prev_u, Op.bitwise_or)
                    else:
                        nc.vector.tensor_tensor(acc[:], u, acc[:], Op.bitwise_or)

                    # row masks: single OR-reduce along columns (DVE)
                    nc.vector.tensor_reduce(rmask[:, :, f, c], u, axis=X,
                                            op=Op.bitwise_or)

                # prefetch next frame's first chunks before the transposes so
                # their loads aren't queued behind acc-dependent triggers
                for c2 in range(3):
                    issue_seg(f + 1, c2)

                # ---- frame tail: transpose (u16), fold the 128 contributors
                accT = accTp.tile([P, KT, MPER, P], u16, tag="accT")
                a16 = acc[:].bitcast(u16).rearrange("p a b -> p (a b)")
                for k in range(KT):
                    nc.sync.dma_start(accT[:, k],
                                      a16[:, 2048 * k:2048 * (k + 1)],
                                      transpose=True)
                w = 64
                while w >= 1:
                    nc.vector.tensor_tensor(accT[:, :, :, 0:w],
                                            accT[:, :, :, 0:w],
                                            accT[:, :, :, w:2 * w],
                                            Op.bitwise_or)
                    w //= 2
                # compact: CMX[q, pl, f, klo, m] <- accT[q, (pl,klo), m, 0]
                nc.vector.tensor_copy(
                    CMX[:, :, f, :, :],
                    accT[:, :, :, 0].rearrange("q (pl klo) m -> q pl klo m",
                                               pl=2, klo=2))

            # ================= extraction =================
            # --- Y side ---
            # rm u16 view: [p, pl, f, c, h]  (h = u16 half; bit b32 = 16h + j)
            rm_e = rmask[:].bitcast(u16).rearrange(
                "p pl f (c h) -> p (pl f) c h", c=CH, h=2)
            Ey = xp.tile([P, 2, TL, 16, CH, 2], i16, tag="ey")
            for j in range(16):
                nc.vector.tensor_scalar(
                    Ey[:, :, :, j].rearrange("p pl f c h -> p (pl f) c h").bitcast(u16),
                    rm_e, j, 1, Op.logical_shift_right, Op.bitwise_and)

            ey_flat = Ey[:].rearrange("p pl f j c h -> p (pl f j) c h")
            CY = xp.tile([P, 64, CH, 2], i16, tag="cy")
            S = smallp.tile([P, 384], i16)
            # Y block layout: col = t*128 + h*64 + (pl f j)  (h-major for output DMAs)
            Sy = S[:, 0:256].rearrange("p (t h a) -> p t h a", t=2, h=2, a=64)
            # ymin: min over c of E*(v-BIG)+BIG
            nc.vector.tensor_tensor(CY[:], ey_flat, c_ty_mb[:], Op.mult)
            nc.vector.tensor_scalar(CY[:], CY[:], BIG, None, Op.add)
            nc.vector.tensor_reduce(Sy[:, 0], CY[:].rearrange("p a c h -> p h a c"),
                                    axis=X, op=Op.min)
            # ymax(+1): max over c of E*(v+1)
            nc.vector.tensor_tensor(CY[:], ey_flat, c_ty_p1[:], Op.mult)
            nc.vector.tensor_reduce(Sy[:, 1], CY[:].rearrange("p a c h -> p h a c"),
                                    axis=X, op=Op.max)

            # --- X side ---
            cmx_flat = CMX[:].rearrange("q pl f klo m -> q (pl f) (klo m)")
            Ex = xp.tile([P, 4, 16, 32], i16, tag="ex")
            for j in range(16):
                nc.vector.tensor_scalar(Ex[:, :, j].bitcast(u16), cmx_flat,
                                        j, 1, Op.logical_shift_right, Op.bitwise_and)
            ex_flat = Ex[:].rearrange("q a j km -> q (a j) km")
            CXt = xp.tile([P, 64, 32], i16, tag="cx")
            nc.vector.tensor_tensor(CXt[:], ex_flat, c_tx_mb[:], Op.mult)
            nc.vector.tensor_scalar(CXt[:], CXt[:], BIG, None, Op.add)
            nc.vector.tensor_reduce(S[:, 256:320], CXt[:], axis=X, op=Op.min)
            nc.vector.tensor_tensor(CXt[:], ex_flat, c_tx_p1[:], Op.mult)
            nc.vector.tensor_reduce(S[:, 320:384], CXt[:], axis=X, op=Op.max)

            # --- partition fold: 3 transposes + reduces ---
            ST = smallp.tile([P, 3, 128], i16)
            for t in range(3):
                eng = nc.scalar if t % 2 else nc.sync
                eng.dma_start(ST[:, t], S[:, 128 * t:128 * (t + 1)],
                              transpose=True)
            # Y: rows (pl f j h); reduce over all 128 contributors
            FY = smallp.tile([P, 2], i16)
            nc.vector.tensor_reduce(FY[:, 0:1], ST[:, 0], axis=X, op=Op.min)
            nc.vector.tensor_reduce(FY[:, 1:2], ST[:, 1], axis=X, op=Op.max)
            # X: rows 0:64 = min (pl f j), 64:128 = max; contributors split by
            # parity g = q&1 (halfword h2 = g); output cols = g
            FX = smallp.tile([P, 2], i16)
            nc.vector.tensor_reduce(
                FX[0:64, :], ST[0:64, 2].rearrange("p (x g) -> p g x", g=2),
                axis=X, op=Op.min)
            nc.vector.tensor_reduce(
                FX[64:128, :], ST[64:128, 2].rearrange("p (x g) -> p g x", g=2),
                axis=X, op=Op.max)

            # --- fixups in f32 ---
            # mins: v==BIG (absent) -> 2147483648.0 ; maxes: v-1 == -1 -> -2^31
            BY = smallp.tile([P, 2], f32)
            BX = smallp.tile([P, 2], f32)
            fy = smallp.tile([P, 2], f32)
            fx = smallp.tile([P, 2], f32)
            nc.gpsimd.tensor_copy(BY[:], FY[:])
            nc.gpsimd.tensor_copy(BX[:], FX[:])
            nc.gpsimd.tensor_scalar(BY[:, 1:2], BY[:, 1:2], 1, 0, Op.subtract, Op.add)
            nc.gpsimd.tensor_scalar(BX[64:128, :], BX[64:128, :], 1, 0,
                                    Op.subtract, Op.add)
            nc.gpsimd.tensor_scalar(fy[:, 0:1], BY[:, 0:1], 32767.0, 2147450880.0,
                                    Op.is_equal, Op.mult)
            nc.gpsimd.tensor_scalar(fy[:, 1:2], BY[:, 1:2], -1.0, -2147483647.0,
                                    Op.is_equal, Op.mult)
            nc.gpsimd.tensor_scalar(fx[0:64, :], BX[0:64, :], 32767.0, 2147450880.0,
                                    Op.is_equal, Op.mult)
            nc.gpsimd.tensor_scalar(fx[64:128, :], BX[64:128, :], -1.0, -2147483647.0,
                                    Op.is_equal, Op.mult)
            nc.gpsimd.tensor_tensor(BY[:], BY[:], fy[:], Op.add)
            nc.gpsimd.tensor_tensor(BX[:], BX[:], fx[:], Op.add)

            # --- output DMAs ---
            # boxes[f, n, k]: k: 0 xmin, 1 ymin, 2 xmax, 3 ymax
            # Y rows p = h*64 + (pl*2+f)*16 + j ; n: pl=0: 31-16h-j, pl=1: 32+16h+j
            outn = [0]

            def out_dma(dst, src):
                eng = nc.scalar if outn[0] % 2 else nc.sync
                outn[0] += 1
                eng.dma_start(dst, src)

            for col, k in [(0, 1), (1, 3)]:
                for h in range(2):
                    base = h * 64
                    out_dma(dram_ap(boxes_out, k + 4 * (31 - 16 * h),
                                    [(256, TL), (-4, 16)]),
                            BY[base:base + 32, col:col + 1])
                    out_dma(dram_ap(boxes_out, k + 4 * (32 + 16 * h),
                                    [(256, TL), (4, 16)]),
                            BY[base + 32:base + 64, col:col + 1])
            # X rows p = base + (pl*2+f)*16+j ; n: pl=0: 31-16g-j, pl=1: 32+16g+j
            for base, k in [(0, 0), (64, 2)]:
                for g in range(2):
                    out_dma(dram_ap(boxes_out, k + 4 * (31 - 16 * g),
                                    [(256, TL), (-4, 16)]),
                            BX[base:base + 32, g:g + 1])
                    out_dma(dram_ap(boxes_out, k + 4 * (32 + 16 * g),
                                    [(256, TL), (4, 16)]),
                            BX[base + 32:base + 64, g:g + 1])

    nc.finalize()
    if split_waits:
        _split_excess_waits(nc, mybir)
    return nc, tables


def _split_excess_waits(nc, mybir):
    """Hoist extra sem waits onto preceding NoOps.

    This walrus build rejects instructions carrying more sync-wait
    conditions than their ISA encoding holds (1 for TPB_CTRL ops and for
    Pool/core_v2 compute ops; 2 elsewhere, conservatively). Semantics are
    identical with the waits split onto dedicated NoOps just before the
    instruction.
    """
    n_split = 0
    for f in nc.m.functions:
        for bb in f.blocks:
            newl = []
            for ins in bb.instructions:
                si = ins.sync_info
                max_waits = 1
                if si and si.on_wait and len(si.on_wait) > max_waits:
                    waits = list(si.on_wait)
                    for j, w in enumerate(waits[max_waits:]):
                        nop = mybir.InstNoOp(
                            name=f"{ins.name}-w{j}", ins=[], outs=[],
                            engine=ins.engine,
                            sync_info=mybir.SyncInfo(on_wait=[w], on_update=[]))
                        newl.append(nop)
                        n_split += 1
                    ins.sync_info = mybir.SyncInfo(on_wait=waits[:max_waits],
                                                   on_update=si.on_update)
                newl.append(ins)
            bb.instructions = newl
    return n_split


def _get_program(TL, H, W, reps=1):
    key = (TL, H, W, reps)
    if key not in _BUILD_CACHE:
        _BUILD_CACHE[key] = _build_program(TL, H, W, reps=reps)
    return _BUILD_CACHE[key]


def kernel(segmentation, num_instances=None, **_ignored):
    from concourse.bass_utils import run_bass_kernel_spmd

    seg = np.asarray(segmentation)
    T, H, W = seg.shape
    assert T % _NCORES == 0
    TL = T // _NCORES
    nc, tables = _get_program(TL, H, W)

    seg = np.ascontiguousarray(seg, dtype=np.int32)
    in_maps = [{"seg": seg[i * TL:(i + 1) * TL], **tables}
               for i in range(_NCORES)]
    res = run_bass_kernel_spmd(nc, in_maps, list(range(_NCORES)))
    out = np.concatenate([res.results[i]["boxes"] for i in range(_NCORES)], axis=0)
    return out.astype(np.float32)


# revision 29
# speedup vs baseline: 1.0172x; 1.0172x over previous
"""Trainium2 Bass kernel for nn_BoxesFromMasks (per-frame segment bounding boxes).

Algorithm (per core, data-parallel over frames, TL=2 frames/core):
  Build per-pixel 64-bit one-hot bitmasks (2 u32 planes) of the instance id via
  the exponent-bit trick (ACT builds the f32 bit pattern of 2^k as an int, a
  second ACT converts value->u32, truncating out-of-range ids to 0):
    lo plane: id s in [0,32)  -> bit (31-s)
    hi plane: id s in [32,64) -> bit (s-32)
  Row masks:  OR-tree each 128-row chunk along columns (DVE), 16-wide leftovers
              folded once at extraction time.
  Col masks:  OR-accumulate chunks into acc[128,2,W]; pre-fold partitions
              128->64; DMA-transpose (u16); OR-tree the 64 contributors.
  Extraction (batched, no DRAM bounce): 16 u16 shift ops expand bits to
  E-tables, constant value-tables select coordinates via i16 mult/add, strided
  tensor_reduce min/max, one 3x128 transpose fold, and negative-stride output
  DMAs undo the bit-order permutation.
"""

import numpy as np

_T, _H, _W, _N = 16, 1024, 2048, 64
_NCORES = 8

_BUILD_CACHE = {}


def _build_program(TL, H, W, split_waits=True, reps=1):
    from contextlib import ExitStack

    import bass_rust
    import concourse.bass as bass
    import concourse.tile as tile
    import concourse.mybir as mybir
    from concourse.alu_op_type import AluOpType as Op

    f32 = mybir.dt.float32
    i32 = mybir.dt.int32
    u32 = mybir.dt.uint32
    u16 = mybir.dt.uint16
    i16 = mybir.dt.int16
    Copy = mybir.ActivationFunctionType.Copy
    X = mybir.AxisListType.X

    P = 128
    CH = H // P                   # row chunks per frame (8)
    KT = 4                        # transpose calls per frame (each 2048 u16 cols)
    MPER = 16                     # 128-col blocks per transpose call
    NSEG = 2                      # seg DMA splits per chunk
    BIG = 32767
    assert TL == 2 and CH == 8 and W == 2048

    # ---- constant value tables (i16) ----
    pp = np.arange(P)
    # Y: value v(p, c) = 128c + p ; table shape [P, 64(pl f j), CH, 2h]
    yv = (128 * np.arange(CH)[None, :] + pp[:, None]).astype(np.int64)   # [P, CH]
    ty_mb = np.broadcast_to((yv - BIG)[:, None, :, None],
                            (P, 64, CH, 2)).astype(np.int16)
    ty_p1 = np.broadcast_to((yv + 1)[:, None, :, None],
                            (P, 64, CH, 2)).astype(np.int16)
    # X: value v(q, klo, m) = klo*1024 + 64m + (q>>1) ; table [P, 64(pl f j), 32(klo m)]
    klo = np.arange(2)
    mm = np.arange(MPER)
    xv = ((klo[:, None] * 1024 + 64 * mm[None, :]).reshape(-1)[None, :]
          + (pp[:, None] // 2)).astype(np.int64)                         # [P, 32]
    tx_mb = np.broadcast_to((xv - BIG)[:, None, :], (P, 64, 32)).astype(np.int16)
    tx_p1 = np.broadcast_to((xv + 1)[:, None, :], (P, 64, 32)).astype(np.int16)

    tables = {"ty_mb": ty_mb, "ty_p1": ty_p1, "tx_mb": tx_mb, "tx_p1": tx_p1}

    nc = bass.Bass()
    seg_in = nc.dram_tensor("seg", [TL, H, W], i32, kind="ExternalInput")
    boxes_out = nc.dram_tensor("boxes", [TL, 64, 4], f32, kind="ExternalOutput")
    d_tabs = {n: nc.dram_tensor(n, list(t.shape), i16, kind="ExternalInput")
              for n, t in tables.items()}

    def dram_ap(t, offset_elems, dims):
        """Manual DRAM AP: dims = [(stride_elems, count), ...]."""
        a2 = t[:].copy()
        a2.offset = offset_elems
        a2.ap = bass_rust.VecI64Pair([[s, n] for s, n in dims])
        return a2

    with tile.TileContext(nc) as tc, ExitStack() as ctx:
        constp = ctx.enter_context(tc.tile_pool(name="consts", bufs=1))
        segp = ctx.enter_context(tc.tile_pool(name="segp", bufs=3))
        ep = ctx.enter_context(tc.tile_pool(name="ep", bufs=3))
        accp = ctx.enter_context(tc.tile_pool(name="accp", bufs=2))
        accTp = ctx.enter_context(tc.tile_pool(name="accTp", bufs=2))
        rmp = ctx.enter_context(tc.tile_pool(name="rmp", bufs=2))
        xp = ctx.enter_context(tc.tile_pool(name="xp", bufs=2))
        smallp = ctx.enter_context(tc.tile_pool(name="smallp", bufs=2))

        c_ty_mb = constp.tile([P, 64, CH, 2], i16)
        c_ty_p1 = constp.tile([P, 64, CH, 2], i16)
        c_tx_mb = constp.tile([P, 64, 32], i16)
        c_tx_p1 = constp.tile([P, 64, 32], i16)
        const_loaded = [False]

        def load_consts():
            if const_loaded[0]:
                return
            const_loaded[0] = True
            for t, n in [(c_ty_mb, "ty_mb"), (c_ty_p1, "ty_p1"),
                         (c_tx_mb, "tx_mb"), (c_tx_p1, "tx_p1")]:
                nc.scalar.dma_start(t[:], d_tabs[n][:])

        for _rep in range(reps):
            # rmask: [p, pl, f, c] u32 (pl-major for contiguous planes)
            rmask = rmp.tile([P, 2, TL, CH], u32, tag="rmask")
            # CMX: [q, pl, f, klo, m] u16 (compacted column masks)
            CMX = xp.tile([P, 2, TL, 2, MPER], u16, tag="cmx")

            # ================= main loop =================
            # seg tiles are created and their loads issued ahead of use so
            # next-frame loads precede this frame's transposes on the queue
            seg_tiles = {}

            def issue_seg(f, c):
                if f >= TL or (f, c) in seg_tiles:
                    return
                s = segp.tile([P, W], i32, tag="seg")
                nseg = 4 if (f == 0 and c == 0) else NSEG
                rows = P // nseg
                for k in range(nseg):
                    nc.sync.dma_start(
                        s[rows * k:rows * (k + 1), :],
                        seg_in[f, c * P + rows * k:c * P + rows * (k + 1), :])
                seg_tiles[(f, c)] = s

            for c2 in range(3):
                issue_seg(0, c2)
            load_consts()

            for f in range(TL):
                acc = accp.tile([P, 2, W], u32)
                prev_u = None
                for c in range(CH):
                    issue_seg(f, c)
                    s = seg_tiles.pop((f, c))

                    e = ep.tile([P, 2, W], i32)
                    # lo: bitpattern of 2^(31-s) = (158-s)<<23 ; hi: 2^(s-32) = (s+95)<<23
                    # first chunk of the kernel: split build/cast into partition
                    # halves so the pipeline fills ~4us sooner
                    halves = ([(0, 64), (64, 128)] if (f == 0 and c == 0)
                              else [(0, 128)])
                    for p0, p1 in halves:
                        nc.scalar.activation(e[p0:p1, 0, :], s[p0:p1], Copy,
                                             bias=1325400064.0, scale=-8388608.0)
                        nc.gpsimd.tensor_scalar(e[p0:p1, 1, :], s[p0:p1],
                                                8388608, 796917760,
                                                Op.mult, Op.add)
                        nc.scalar.activation(e[p0:p1].bitcast(u32),
                                             e[p0:p1].bitcast(f32), Copy)
                    u = e[:].bitcast(u32)  # cast in place

                    # column accumulate (DVE; only DVE has integer bitwise ops)
                    if c == 0:
                        prev_u = u
                    elif c == 1:
                        nc.vector.tensor_tensor(acc[:], u, prev_u, Op.bitwise_or)
                    else:
                        nc.vector.tensor_tensor(acc[:], u, acc[:], Op.bitwise_or)

                    # row masks: single OR-reduce along columns (DVE)
                    nc.vector.tensor_reduce(rmask[:, :, f, c], u, axis=X,
                                            op=Op.bitwise_or)

                # prefetch next frame's first chunks before the transposes so
                # their loads aren't queued behind acc-dependent triggers
                for c2 in range(3):
                    issue_seg(f + 1, c2)

                # ---- frame tail: transpose (u16), fold the 128 contributors
                accT = accTp.tile([P, KT, MPER, P], u16, tag="accT")
                a16 = acc[:].bitcast(u16).rearrange("p a b -> p (a b)")
                for k in range(KT):
                    nc.sync.dma_start(accT[:, k],
                                      a16[:, 2048 * k:2048 * (k + 1)],
                                      transpose=True)
                w = 64
                while w >= 2:
                    nc.vector.tensor_tensor(accT[:, :, :, 0:w],
                                            accT[:, :, :, 0:w],
                                            accT[:, :, :, w:2 * w],
                                            Op.bitwise_or)
                    w //= 2
                # last fold level writes CMX[q, pl, f, klo, m] directly
                nc.vector.tensor_tensor(
                    CMX[:, :, f, :, :],
                    accT[:, :, :, 0].rearrange("q (pl klo) m -> q pl klo m",
                                               pl=2, klo=2),
                    accT[:, :, :, 1].rearrange("q (pl klo) m -> q pl klo m",
                                               pl=2, klo=2),
                    Op.bitwise_or)

            # ================= extraction =================
            # --- Y side ---
            # rm u16 view: [p, pl, f, c, h]  (h = u16 half; bit b32 = 16h + j)
            rm_e = rmask[:].bitcast(u16).rearrange(
                "p pl f (c h) -> p (pl f) c h", c=CH, h=2)
            Ey = xp.tile([P, 2, TL, 16, CH, 2], i16, tag="ey")
            for j in range(16):
                nc.vector.tensor_scalar(
                    Ey[:, :, :, j].rearrange("p pl f c h -> p (pl f) c h").bitcast(u16),
                    rm_e, j, 1, Op.logical_shift_right, Op.bitwise_and)

            ey_flat = Ey[:].rearrange("p pl f j c h -> p (pl f j) c h")
            CY = xp.tile([P, 64, CH, 2], i16, tag="cy")
            S = smallp.tile([P, 384], i16)
            # Y block layout: col = t*128 + h*64 + (pl f j)  (h-major for output DMAs)
            Sy = S[:, 0:256].rearrange("p (t h a) -> p t h a", t=2, h=2, a=64)
            # ymin: min over c of E*(v-BIG)+BIG
            nc.vector.tensor_tensor(CY[:], ey_flat, c_ty_mb[:], Op.mult)
            nc.vector.tensor_reduce(Sy[:, 0], CY[:].rearrange("p a c h -> p h a c"),
                                    axis=X, op=Op.min)
            # ymax(+1): max over c of E*(v+1)
            nc.vector.tensor_tensor(CY[:], ey_flat, c_ty_p1[:], Op.mult)
            nc.vector.tensor_reduce(Sy[:, 1], CY[:].rearrange("p a c h -> p h a c"),
                                    axis=X, op=Op.max)

            # --- X side ---
            cmx_flat = CMX[:].rearrange("q pl f klo m -> q (pl f) (klo m)")
            Ex = xp.tile([P, 4, 16, 32], i16, tag="ex")
            for j in range(16):
                nc.vector.tensor_scalar(Ex[:, :, j].bitcast(u16), cmx_flat,
                                        j, 1, Op.logical_shift_right, Op.bitwise_and)
            ex_flat = Ex[:].rearrange("q a j km -> q (a j) km")
            CXt = xp.tile([P, 64, 32], i16, tag="cx")
            nc.vector.tensor_tensor(CXt[:], ex_flat, c_tx_mb[:], Op.mult)
            nc.vector.tensor_reduce(S[:, 256:320], CXt[:], axis=X, op=Op.min)
            nc.vector.tensor_tensor(CXt[:], ex_flat, c_tx_p1[:], Op.mult)
            nc.vector.tensor_reduce(S[:, 320:384], CXt[:], axis=X, op=Op.max)

            # --- partition fold: 3 transposes + reduces ---
            ST = smallp.tile([P, 3, 128], i16)
            for t in range(3):
                eng = nc.scalar if t % 2 else nc.sync
                eng.dma_start(ST[:, t], S[:, 128 * t:128 * (t + 1)],
                              transpose=True)
            # Y: rows (pl f j h); reduce over all 128 contributors
            FY = smallp.tile([P, 2], i16)
            nc.vector.tensor_reduce(FY[:, 0:1], ST[:, 0], axis=X, op=Op.min)
            nc.vector.tensor_reduce(FY[:, 1:2], ST[:, 1], axis=X, op=Op.max)
            # X: rows 0:64 = min (pl f j), 64:128 = max; contributors split by
            # parity g = q&1 (halfword h2 = g); output cols = g
            FX = smallp.tile([P, 2], i16)
            nc.vector.tensor_reduce(
                FX[0:64, :], ST[0:64, 2].rearrange("p (x g) -> p g x", g=2),
                axis=X, op=Op.min)
            nc.vector.tensor_reduce(
                FX[64:128, :], ST[64:128, 2].rearrange("p (x g) -> p g x", g=2),
                axis=X, op=Op.max)

            # --- fixups in f32 ---
            # mins: v==BIG (absent) -> 2147483648.0 ; maxes: v-1 == -1 -> -2^31
            BY = smallp.tile([P, 2], f32)
            BX = smallp.tile([P, 2], f32)
            fy = smallp.tile([P, 2], f32)
            fx = smallp.tile([P, 2], f32)
            nc.gpsimd.tensor_copy(BY[:], FY[:])
            nc.gpsimd.tensor_copy(BX[:], FX[:])
            nc.gpsimd.tensor_scalar(BY[:, 0:1], BY[:, 0:1], 32767.0, 0.0,
                                    Op.add, Op.add)
            nc.gpsimd.tensor_scalar(BY[:, 1:2], BY[:, 1:2], 1, 0, Op.subtract, Op.add)
            nc.gpsimd.tensor_scalar(BX[0:64, :], BX[0:64, :], 32767.0, 0.0,
                                    Op.add, Op.add)
            nc.gpsimd.tensor_scalar(BX[64:128, :], BX[64:128, :], 1, 0,
                                    Op.subtract, Op.add)
            nc.gpsimd.tensor_scalar(fy[:, 0:1], BY[:, 0:1], 32767.0, 2147450880.0,
                                    Op.is_equal, Op.mult)
            nc.gpsimd.tensor_scalar(fy[:, 1:2], BY[:, 1:2], -1.0, -2147483647.0,
                                    Op.is_equal, Op.mult)
            nc.gpsimd.tensor_scalar(fx[0:64, :], BX[0:64, :], 32767.0, 2147450880.0,
                                    Op.is_equal, Op.mult)
            nc.gpsimd.tensor_scalar(fx[64:128, :], BX[64:128, :], -1.0, -2147483647.0,
                                    Op.is_equal, Op.mult)
            nc.gpsimd.tensor_tensor(BY[:], BY[:], fy[:], Op.add)
            nc.gpsimd.tensor_tensor(BX[:], BX[:], fx[:], Op.add)

            # --- output DMAs ---
            # boxes[f, n, k]: k: 0 xmin, 1 ymin, 2 xmax, 3 ymax
            # Y rows p = h*64 + (pl*2+f)*16 + j ; n: pl=0: 31-16h-j, pl=1: 32+16h+j
            outn = [0]

            def out_dma(dst, src):
                eng = nc.scalar if outn[0] % 2 else nc.sync
                outn[0] += 1
                eng.dma_start(dst, src)

            for col, k in [(0, 1), (1, 3)]:
                for h in range(2):
                    base = h * 64
                    out_dma(dram_ap(boxes_out, k + 4 * (31 - 16 * h),
                                    [(256, TL), (-4, 16)]),
                            BY[base:base + 32, col:col + 1])
                    out_dma(dram_ap(boxes_out, k + 4 * (32 + 16 * h),
                                    [(256, TL), (4, 16)]),
                            BY[base + 32:base + 64, col:col + 1])
            # X rows p = base + (pl*2+f)*16+j ; n: pl=0: 31-16g-j, pl=1: 32+16g+j
            for base, k in [(0, 0), (64, 2)]:
                for g in range(2):
                    out_dma(dram_ap(boxes_out, k + 4 * (31 - 16 * g),
                                    [(256, TL), (-4, 16)]),
                            BX[base:base + 32, g:g + 1])
                    out_dma(dram_ap(boxes_out, k + 4 * (32 + 16 * g),
                                    [(256, TL), (4, 16)]),
                            BX[base + 32:base + 64, g:g + 1])

    nc.finalize()
    if split_waits:
        _split_excess_waits(nc, mybir)
    return nc, tables


def _split_excess_waits(nc, mybir):
    """Hoist extra sem waits onto preceding NoOps.

    This walrus build rejects instructions carrying more sync-wait
    conditions than their ISA encoding holds (1 for TPB_CTRL ops and for
    Pool/core_v2 compute ops; 2 elsewhere, conservatively). Semantics are
    identical with the waits split onto dedicated NoOps just before the
    instruction.
    """
    n_split = 0
    for f in nc.m.functions:
        for bb in f.blocks:
            newl = []
            for ins in bb.instructions:
                si = ins.sync_info
                max_waits = 1
                if si and si.on_wait and len(si.on_wait) > max_waits:
                    waits = list(si.on_wait)
                    for j, w in enumerate(waits[max_waits:]):
                        nop = mybir.InstNoOp(
                            name=f"{ins.name}-w{j}", ins=[], outs=[],
                            engine=ins.engine,
                            sync_info=mybir.SyncInfo(on_wait=[w], on_update=[]))
                        newl.append(nop)
                        n_split += 1
                    ins.sync_info = mybir.SyncInfo(on_wait=waits[:max_waits],
                                                   on_update=si.on_update)
                newl.append(ins)
            bb.instructions = newl
    return n_split


def _get_program(TL, H, W, reps=1):
    key = (TL, H, W, reps)
    if key not in _BUILD_CACHE:
        _BUILD_CACHE[key] = _build_program(TL, H, W, reps=reps)
    return _BUILD_CACHE[key]


def kernel(segmentation, num_instances=None, **_ignored):
    from concourse.bass_utils import run_bass_kernel_spmd

    seg = np.asarray(segmentation)
    T, H, W = seg.shape
    assert T % _NCORES == 0
    TL = T // _NCORES
    nc, tables = _get_program(TL, H, W)

    seg = np.ascontiguousarray(seg, dtype=np.int32)
    in_maps = [{"seg": seg[i * TL:(i + 1) * TL], **tables}
               for i in range(_NCORES)]
    res = run_bass_kernel_spmd(nc, in_maps, list(range(_NCORES)))
    out = np.concatenate([res.results[i]["boxes"] for i in range(_NCORES)], axis=0)
    return out.astype(np.float32)
